# revision 26
# baseline (speedup 1.0000x reference)
"""Trainium2 Bass kernel for nn_Dynamic_Atten_Autoencoder (GNN message passing).

Self-contained: accepts FULL inputs, shards across 8 NeuronCores internally,
returns the FULL output tuple (hiden_emb, h, ret, ret_a).

Strategy (row/dst partition parallel):
  - 512 node rows per core; edges partitioned by dst; GAT softmax without the
    (mathematically redundant) segment-max shift.
  - Per-edge gathers via the SWDGE dma_gather custom instruction (<=1024
    indices per instruction — descriptor-ring limit) from fp16 node tables;
    src-indexed tables are AllGather'd, dst-indexed tables stay core-local.
  - Segment softmax-sum via one-hot H^T matmuls on the tensor engine
    (lhsT = per-edge values*exp(e), rhs = one-hot dst columns).
  - GATv2 layer-2 edge scores use the exact decomposition
      att^T lrelu(S) = 0.6*att^T S + 0.4*sum_w sign(att_w)*|att_w*S_w|
    so the per-edge 512-dim work is one fp16 add + two |.|-reduces on DVE.
  - Dense adj matmuls produce z^T directly (lhsT = allgathered z1 rows,
    rhs = adj^T chunks) so downstream matmuls need no big transposes.
  - Readout/bilinear are emitted between the t2 AllGather and the L2 edge
    phase to fill the collective's dead time; the h1 AllGather is split in
    column halves so the h matmul overlaps the second half.
"""

import numpy as np

N = 4096
R = 512            # rows (nodes) per core
DIN = 512
DOUT = 64
E = 65536
NC = 8
EP = 9216          # padded edges per core (max actual is 8336)
NCHUNK = EP // 128     # 72
BLK = 8                # chunks per gather block (1024 edges = the SWDGE
NBLK = NCHUNK // BLK   # 9   per-instruction descriptor-ring limit)
EB = BLK * 128         # 1024 edges per block
ADJ_SCALE = 4096.0     # fp16 subnormal avoidance for adj values ~U(0, 1/4096)

_CACHE = {}


def _wrap_idx(idx: np.ndarray) -> np.ndarray:
    """Wrap indices into the [128, EP//16] int16 layout dma_gather expects:
    index i lives at [i % 16, i // 16], replicated across the 8 Q7 cores."""
    w = np.zeros((128, EP // 16), np.int16)
    base = idx.astype(np.int16).reshape(EP // 16, 16).T  # [16, cols]
    for rep in range(8):
        w[rep * 16:(rep + 1) * 16] = base
    return w


def _build_program():
    import concourse.bass as bass
    import concourse.tile as tile
    from concourse import bacc, mybir, library_config
    from concourse.masks import make_identity

    f32 = mybir.dt.float32
    f16 = mybir.dt.float16
    i16 = mybir.dt.int16
    AF = mybir.ActivationFunctionType
    OP = mybir.AluOpType
    X = mybir.AxisListType.X

    nc = bacc.Bacc("TRN2", target_bir_lowering=False, debug=False, num_devices=NC)

    # ---------------- DRAM declarations ----------------
    featT = nc.dram_tensor("featT", [DIN, R], f32, kind="ExternalInput")
    feataT = nc.dram_tensor("feataT", [DIN, R], f32, kind="ExternalInput")
    adjT16 = nc.dram_tensor("adjT16", [N, R], f16, kind="ExternalInput")
    gnT16 = nc.dram_tensor("gnT16", [N, R], f16, kind="ExternalInput")
    srcw = nc.dram_tensor("srcw", [128, EP // 16], i16, kind="ExternalInput")
    dstlw = nc.dram_tensor("dstlw", [128, EP // 16], i16, kind="ExternalInput")
    dstl = nc.dram_tensor("dstl", [128, NCHUNK, 1], f16, kind="ExternalInput")
    emask = nc.dram_tensor("emask", [128, NCHUNK, 1], f16, kind="ExternalInput")
    Wl1h = nc.dram_tensor("Wl1h", [DIN, DOUT], f16, kind="ExternalInput")
    Wr1h = nc.dram_tensor("Wr1h", [DIN, DOUT], f16, kind="ExternalInput")
    att1r = nc.dram_tensor("att1r", [128, DOUT], f16, kind="ExternalInput")
    W2sl = nc.dram_tensor("W2sl", [DOUT, DIN], f16, kind="ExternalInput")
    W2sr = nc.dram_tensor("W2sr", [DOUT, DIN], f16, kind="ExternalInput")
    cl6 = nc.dram_tensor("cl6", [DOUT, 1], f16, kind="ExternalInput")
    cr6 = nc.dram_tensor("cr6", [DOUT, 1], f16, kind="ExternalInput")
    Wl2h = nc.dram_tensor("Wl2h", [DOUT, DIN], f16, kind="ExternalInput")
    WbTh = nc.dram_tensor("WbTh", [DOUT, DOUT], f16, kind="ExternalInput")
    bbcol = nc.dram_tensor("bbcol", [128, 1], f32, kind="ExternalInput")
    iota512 = nc.dram_tensor("iota512", [128, DIN], f16, kind="ExternalInput")
    nposc = _CACHE["npos"]  # python int baked into the program

    # internal DRAM (per-core locals + allgather outputs)
    t1loc = nc.dram_tensor("t1loc", [R, 128], f16)
    t1bloc = nc.dram_tensor("t1bloc", [R, 128], f16)
    z1loc = nc.dram_tensor("z1loc", [R, 128], f16)
    embloc = nc.dram_tensor("embloc", [R, 256], f16)
    t2loc = nc.dram_tensor("t2loc", [R, DIN], f16)
    t2bloc = nc.dram_tensor("t2bloc", [R, 640], f16)
    aggnloc = nc.dram_tensor("aggnloc", [R, DOUT], f16)
    t1full = nc.dram_tensor("t1full", [N, 128], f16, addr_space="Shared")
    z1full = nc.dram_tensor("z1full", [N, 128], f16, addr_space="Shared")
    embfull = nc.dram_tensor("embfull", [N, 256], f16, addr_space="Shared")
    t2full = nc.dram_tensor("t2full", [N, DIN], f16, addr_space="Shared")
    aggnfull = nc.dram_tensor("aggnfull", [N, DOUT], f16, addr_space="Shared")

    hid_out = nc.dram_tensor("hid_out", [R, DOUT], f32, kind="ExternalOutput")
    h_out = nc.dram_tensor("h_out", [R, DIN], f32, kind="ExternalOutput")
    ret_out = nc.dram_tensor("ret_out", [R, 2], f32, kind="ExternalOutput")
    reta_out = nc.dram_tensor("reta_out", [R, 2], f32, kind="ExternalOutput")

    rg = [list(range(NC))]
    MS = lambda m: slice(128 * m, 128 * (m + 1))

    def AG(ins_ap, outs_ap):
        nc.gpsimd.collective_compute(
            "AllGather", mybir.AluOpType.bypass, replica_groups=rg,
            ins=[ins_ap], outs=[outs_ap])

    class _PhaseStop(Exception):
        pass

    import contextlib
    MAXPH = _CACHE.get("max_phase", 99)

    def gate(n):
        if MAXPH < n:
            raise _PhaseStop

    with tile.TileContext(nc) as tc:
        from contextlib import ExitStack
        with contextlib.suppress(_PhaseStop), ExitStack() as ctx:
            wp = ctx.enter_context(tc.tile_pool(name="wp", bufs=1))
            persist = ctx.enter_context(tc.tile_pool(name="persist", bufs=1))
            ldp = ctx.enter_context(tc.tile_pool(name="ldp", bufs=2))
            work = ctx.enter_context(tc.tile_pool(name="work", bufs=3))
            gath = ctx.enter_context(tc.tile_pool(name="gath", bufs=2))
            htp = ctx.enter_context(tc.tile_pool(name="htp", bufs=1))
            vp = ctx.enter_context(tc.tile_pool(name="vp", bufs=2))
            eep = ctx.enter_context(tc.tile_pool(name="eep", bufs=2))
            adjp = ctx.enter_context(tc.tile_pool(name="adjp", bufs=1))
            gnp = ctx.enter_context(tc.tile_pool(name="gnp", bufs=1))

            nc.gpsimd.load_library(library_config.mlp)

            # ---- constants / weights into SBUF ----
            def _load(pool, dram, shape, dtype):
                t = pool.tile(shape, dtype, tag=dram.name)
                nc.sync.dma_start(t[:], dram.ap())
                return t

            Wl1t = wp.tile([128, 4, DOUT], f16, tag="Wl1t")
            nc.sync.dma_start(Wl1t[:], Wl1h.ap().rearrange("(k p) d -> p k d", p=128))
            Wr1t = wp.tile([128, 4, DOUT], f16, tag="Wr1t")
            nc.sync.dma_start(Wr1t[:], Wr1h.ap().rearrange("(k p) d -> p k d", p=128))
            att1t = _load(wp, att1r, [128, DOUT], f16)
            W2slt = _load(wp, W2sl, [DOUT, DIN], f16)
            W2srt = _load(wp, W2sr, [DOUT, DIN], f16)
            cl6t = _load(wp, cl6, [DOUT, 1], f16)
            cr6t = _load(wp, cr6, [DOUT, 1], f16)
            Wl2t = _load(wp, Wl2h, [DOUT, DIN], f16)
            WbTt = _load(wp, WbTh, [DOUT, DOUT], f16)
            bbt = _load(wp, bbcol, [128, 1], f32)
            iotat = _load(wp, iota512, [128, DIN], f16)
            srct = _load(wp, srcw, [128, EP // 16], i16)
            dstlwt = _load(wp, dstlw, [128, EP // 16], i16)
            dstlt = _load(wp, dstl, [128, NCHUNK, 1], f16)
            emaskt = _load(wp, emask, [128, NCHUNK, 1], f16)
            ident = wp.tile([128, 128], f16, tag="ident")
            make_identity(nc, ident[:])
            idf32 = wp.tile([128, 128], f32, tag="idf32")
            make_identity(nc, idf32[:])

            iotab = iotat[:].rearrange("p (one d) -> p one d", one=1) \
                            .to_broadcast([128, BLK, DIN])

            def build_HT(b, pfx):
                """One-hot H^T block: HT[p, c, d] = (dstl[edge p,c] == d)."""
                HT = htp.tile([128, BLK, DIN], f16, tag="HT",
                              name=f"HT{pfx}{b}")
                nc.vector.tensor_tensor(
                    HT[:], iotab,
                    dstlt[:, BLK * b:BLK * (b + 1), :].to_broadcast(
                        [128, BLK, DIN]),
                    op=OP.is_equal)
                return HT

            # ---- feat^T chunks -> fp16 ----
            gate(1)
            featTh = persist.tile([128, 4, DIN], f16, tag="featTh")
            feataTh = persist.tile([128, 4, DIN], f16, tag="feataTh")
            for kc in range(4):
                for (dram, dst_) in ((featT, featTh), (feataT, feataTh)):
                    l = ldp.tile([128, 4, R], f32, tag="fload")
                    nc.sync.dma_start(l[:, 0, :], dram[MS(kc), :])
                    nc.vector.tensor_copy(dst_[:, kc, :], l[:, 0, :])

            # ---- adjT bulk load (fp16 direct, early) ----
            gate(2)
            adjTh = adjp.tile([128, 32, R], f16, tag="adjTh")
            for g in range(4):
                nc.sync.dma_start(
                    adjTh[:, 8 * g:8 * (g + 1), :],
                    adjT16[1024 * g:1024 * (g + 1), :].rearrange(
                        "(j p) r -> p j r", p=128))

            # ---- T1 = [xl1 | xl1a] (AG'd) and [xr1 | xr1a] (local) ----
            gate(3)
            with tc.tile_pool(name="p1", bufs=4, space="PSUM") as p1:
                for m in range(4):
                    outs = []
                    for (fT, Wt) in ((featTh, Wl1t), (feataTh, Wl1t),
                                     (featTh, Wr1t), (feataTh, Wr1t)):
                        ps = p1.tile([128, DOUT], f32, tag="t1ps")
                        for kc in range(4):
                            nc.tensor.matmul(ps[:], fT[:, kc, MS(m)],
                                             Wt[:, kc, :],
                                             start=(kc == 0), stop=(kc == 3))
                        outs.append(ps)
                    t1s = work.tile([128, 128], f16, tag="t1s", bufs=2)
                    t1bs = work.tile([128, 128], f16, tag="t1bs", bufs=2)
                    nc.vector.tensor_copy(t1s[:, 0:64], outs[0][:])
                    nc.vector.tensor_copy(t1s[:, 64:128], outs[1][:])
                    nc.vector.tensor_copy(t1bs[:, 0:64], outs[2][:])
                    nc.vector.tensor_copy(t1bs[:, 64:128], outs[3][:])
                    nc.sync.dma_start(t1loc[MS(m), :], t1s[:])
                    nc.sync.dma_start(t1bloc[MS(m), :], t1bs[:])
            AG(t1loc.ap(), t1full.ap())

            # ---- L1 edge gathers + scores ----
            gate(4)
            A1 = gath.tile([128, NCHUNK, 128], f16, tag="gslot")
            B1 = gath.tile([128, NCHUNK, 128], f16, tag="gslot")
            for g in range(NBLK):
                isl = slice((EB // 16) * g, (EB // 16) * (g + 1))
                csl = slice(BLK * g, BLK * (g + 1))
                nc.gpsimd.dma_gather(B1[:, csl, :], t1bloc[:, 0:128],
                                     dstlwt[:, isl], EB, EB, 128,
                                     elem_step=128)
            for g in range(NBLK):
                isl = slice((EB // 16) * g, (EB // 16) * (g + 1))
                csl = slice(BLK * g, BLK * (g + 1))
                nc.gpsimd.dma_gather(A1[:, csl, :], t1full[:, 0:128],
                                     srct[:, isl], EB, EB, 128, elem_step=128)
            # scores + aggregation pipelined in two chunk-range halves
            attb = att1t[:].rearrange("p (one d) -> p one d", one=1) \
                           .to_broadcast([128, 2 * 40, DOUT])
            e1 = eep.tile([128, NCHUNK, 2], f32, tag="e1")
            eeh = eep.tile([128, NCHUNK, 2], f16, tag="eeh")
            V1 = vp.tile([128, NCHUNK, 65], f16, tag="vslot")
            V1a = vp.tile([128, NCHUNK, 65], f16, tag="vslot")
            with tc.tile_pool(name="pagg", bufs=2, space="PSUM") as pagg:
                agg1 = pagg.tile([65, R], f32, tag="agg")
                agg1a = pagg.tile([65, R], f32, tag="agg")
                for (c0, c1) in ((0, 32), (32, 72)):
                    nch = c1 - c0
                    cs = slice(c0, c1)
                    # u = A + B (into B; A keeps raw values for aggregation)
                    nc.vector.tensor_tensor(B1[:, cs, :], A1[:, cs, :],
                                            B1[:, cs, :], op=OP.add)
                    # lrelu(u) = max(0.2*u, u)
                    nc.vector.scalar_tensor_tensor(B1[:, cs, :], B1[:, cs, :],
                                                   0.2, B1[:, cs, :],
                                                   op0=OP.mult, op1=OP.max)
                    uview = B1[:, cs, :].rearrange("p c (f d) -> p (c f) d",
                                                   d=DOUT)
                    nc.vector.tensor_tensor(
                        uview, uview,
                        attb[:, 0:2 * nch, :], op=OP.mult)
                    nc.vector.tensor_reduce(
                        e1[:, cs, :].rearrange("p c f -> p (c f)"), uview,
                        axis=X, op=OP.add)
                    nc.scalar.activation(e1[:, cs, :], e1[:, cs, :], AF.Exp)
                    nc.vector.tensor_tensor(
                        e1[:, cs, :], e1[:, cs, :],
                        emaskt[:, cs, :].to_broadcast([128, nch, 2]),
                        op=OP.mult)
                    nc.vector.tensor_copy(eeh[:, cs, :], e1[:, cs, :])
                    nc.vector.tensor_tensor(
                        V1[:, cs, 0:64], A1[:, cs, 0:64],
                        eeh[:, cs, 0:1].to_broadcast([128, nch, 64]),
                        op=OP.mult)
                    nc.vector.tensor_copy(V1[:, cs, 64:65], eeh[:, cs, 0:1])
                    nc.vector.tensor_tensor(
                        V1a[:, cs, 0:64], A1[:, cs, 64:128],
                        eeh[:, cs, 1:2].to_broadcast([128, nch, 64]),
                        op=OP.mult)
                    nc.vector.tensor_copy(V1a[:, cs, 64:65], eeh[:, cs, 1:2])
                    for b in range(c0 // BLK, c1 // BLK):
                        HT = build_HT(b, "L1")
                        for c in range(BLK):
                            cc = BLK * b + c
                            nc.tensor.matmul(agg1[:], V1[:, cc, :],
                                             HT[:, c, :], start=(cc == 0),
                                             stop=(cc == NCHUNK - 1))
                            nc.tensor.matmul(agg1a[:], V1a[:, cc, :],
                                             HT[:, c, :], start=(cc == 0),
                                             stop=(cc == NCHUNK - 1))
                a1 = work.tile([65, R], f32, tag="aggev", bufs=2)
                nc.vector.tensor_copy(a1[:], agg1[:])
                a1a = work.tile([65, R], f32, tag="aggev", bufs=2)
                nc.vector.tensor_copy(a1a[:], agg1a[:])
            # load gn while the DMA engines are otherwise idle
            gnTh = gnp.tile([128, 32, R], f16, tag="gnTh")
            for g in range(4):
                nc.sync.dma_start(
                    gnTh[:, 8 * g:8 * (g + 1), :],
                    gnT16[1024 * g:1024 * (g + 1), :].rearrange(
                        "(j p) r -> p j r", p=128))
            # transpose (num|den) to node rows, normalize per partition
            with tc.tile_pool(name="pt1", bufs=4, space="PSUM") as pt:
                for m in range(4):
                    ev = work.tile([128, 128], f16, tag="z1ev")
                    for (src_, off) in ((a1, 0), (a1a, 64)):
                        tp = pt.tile([128, 65], f32, tag="tp65")
                        nc.tensor.transpose(tp[:], src_[:, MS(m)],
                                            idf32[0:65, 0:65])
                        rsr = work.tile([128, 1], f32, tag="rsr")
                        nc.vector.tensor_scalar_max(rsr[:], tp[:, 64:65], 1e-30)
                        nc.vector.reciprocal(rsr[:], rsr[:])
                        nc.vector.tensor_scalar_mul(ev[:, off:off + 64],
                                                    tp[:, 0:64], rsr[:])
                    nc.sync.dma_start(z1loc[MS(m), :], ev[:])
            AG(z1loc.ap(), z1full.ap())

            # ---- zT = (z1_full^T @ adjT) scaled ----
            gate(6)
            zTh = persist.tile([128, R], f16, tag="zTh")
            with tc.tile_pool(name="pz", bufs=1, space="PSUM") as pz:
                zps = pz.tile([128, R], f32, tag="zps")
                for g in range(8):
                    l = work.tile([128, 4, 128], f16, tag="z1ld", bufs=2)
                    nc.sync.dma_start(
                        l[:], z1full[512 * g:512 * (g + 1), :].rearrange(
                            "(j p) r -> p j r", p=128))
                    for j in range(4):
                        kc = 4 * g + j
                        nc.tensor.matmul(zps[:], l[:, j, :], adjTh[:, kc, :],
                                         start=(kc == 0), stop=(kc == 31))
                nc.vector.tensor_scalar_mul(zTh[:], zps[:], 1.0 / ADJ_SCALE)

            # ---- z node rows: hid output, emb, z/p2l staging ----
            gate(7)
            embf32 = persist.tile([128, 4, 128], f32, tag="embf32")
            zzh = persist.tile([128, 4, DOUT], f16, tag="zzh")
            p2lv = persist.tile([128, 4], f16, tag="p2lv")
            with tc.tile_pool(name="pt2", bufs=2, space="PSUM") as pt, \
                 tc.tile_pool(name="pp2l", bufs=2, space="PSUM") as pp2l:
                for m in range(4):
                    tp = pt.tile([128, 128], f16, tag="tp")
                    nc.tensor.transpose(tp[:], zTh[:, MS(m)], ident[:])
                    zn = work.tile([128, 128], f32, tag="zn", bufs=2)
                    nc.vector.tensor_copy(zn[:], tp[:])
                    nc.sync.dma_start(hid_out[MS(m), :], zn[:, 0:64])
                    nc.vector.tensor_scalar_max(embf32[:, m, :], zn[:], 0.0)
                    nc.vector.tensor_copy(zzh[:, m, :], tp[:, 0:64])
                    pp = pp2l.tile([128, 1], f32, tag="p2lps")
                    nc.tensor.matmul(pp[:], zTh[0:64, MS(m)], cl6t[:],
                                     start=True, stop=True)
                    nc.vector.tensor_copy(p2lv[:, m:m + 1], pp[:])

            # ---- T2 tables: xl2s (AG'd), [xr2s|p2r] (local) ----
            gate(8)
            with tc.tile_pool(name="p2", bufs=2, space="PSUM") as p2, \
                 tc.tile_pool(name="p2s", bufs=2, space="PSUM") as p2s:
                for m in range(4):
                    t2t = work.tile([128, DIN], f16, tag="t2t", bufs=2)
                    ps = p2.tile([128, DIN], f32, tag="t2ps")
                    nc.tensor.matmul(ps[:], zTh[0:64, MS(m)], W2slt[:],
                                     start=True, stop=True)
                    nc.vector.tensor_copy(t2t[:], ps[:])
                    nc.sync.dma_start(t2loc[MS(m), :], t2t[:])
                    t2bt = work.tile([128, 640], f16, tag="t2bt", bufs=2)
                    ps2 = p2.tile([128, DIN], f32, tag="t2ps")
                    nc.tensor.matmul(ps2[:], zTh[0:64, MS(m)], W2srt[:],
                                     start=True, stop=True)
                    pp2 = p2s.tile([128, 1], f32, tag="t2pp")
                    nc.tensor.matmul(pp2[:], zTh[0:64, MS(m)], cr6t[:],
                                     start=True, stop=True)
                    nc.vector.tensor_copy(t2bt[:, 0:512], ps2[:])
                    nc.vector.tensor_copy(t2bt[:, 512:513], pp2[:])
                    nc.vector.memset(t2bt[:, 513:640], 0.0)
                    nc.sync.dma_start(t2bloc[MS(m), :], t2bt[:])
            AG(t2loc.ap(), t2full.ap())
            for m in range(4):
                et = work.tile([128, 256], f16, tag="et", bufs=2)
                nc.vector.tensor_copy(et[:, 0:128], embf32[:, m, :])
                nc.vector.memset(et[:, 128:129], 1.0)
                nc.vector.tensor_copy(et[:, 129:193], zzh[:, m, :])
                nc.vector.tensor_copy(et[:, 193:194], p2lv[:, m:m + 1])
                nc.vector.memset(et[:, 194:256], 0.0)
                nc.sync.dma_start(embloc[MS(m), :], et[:])
            AG(embloc.ap(), embfull.ap())

            # ---- readout (fills the t2/emb AllGather window) ----
            gate(9)
            gh = persist.tile([128, R], f16, tag="gh")
            with tc.tile_pool(name="pr", bufs=4, space="PSUM") as pr:
                rps = [pr.tile([128, 130], f32, tag="rps", name=f"rps{m}")
                       for m in range(4)]
                for g in range(8):
                    ec = work.tile([128, 4, 130], f16, tag="ecld", bufs=2)
                    nc.sync.dma_start(
                        ec[:], embfull[512 * g:512 * (g + 1), 0:130].rearrange(
                            "(j p) r -> p j r", p=128))
                    for j in range(4):
                        kc = 4 * g + j
                        for m in range(4):
                            nc.tensor.matmul(rps[m][:], gnTh[:, kc, MS(m)],
                                             ec[:, j, :],
                                             start=(kc == 0), stop=(kc == 31))
                for m in range(4):
                    rv = work.tile([128, 130], f32, tag="rv", bufs=2)
                    nc.vector.tensor_copy(rv[:], rps[m][:])
                    rsr = work.tile([128, 1], f32, tag="rsr")
                    nc.vector.reciprocal(rsr[:], rv[:, 128:129])
                    gpre = work.tile([128, 128], f32, tag="gpre", bufs=2)
                    nc.vector.tensor_scalar_mul(gpre[:], rv[:, 0:128], rsr[:])
                    sq = work.tile([128, 128], f32, tag="sq", bufs=2)
                    nc.vector.tensor_tensor(sq[:], gpre[:], gpre[:], op=OP.mult)
                    n2 = work.tile([128, 2], f32, tag="n2")
                    nc.vector.tensor_reduce(
                        n2[:], sq[:].rearrange("p (f d) -> p f d", d=64),
                        axis=X, op=OP.add)
                    nc.vector.tensor_scalar_max(n2[:], n2[:], 1e-24)
                    nc.scalar.activation(n2[:], n2[:], AF.Sqrt)
                    nc.vector.reciprocal(n2[:], n2[:])
                    nc.vector.tensor_scalar_mul(gpre[:, 0:64], gpre[:, 0:64],
                                                n2[:, 0:1])
                    nc.vector.tensor_scalar_mul(gpre[:, 64:128],
                                                gpre[:, 64:128], n2[:, 1:2])
                    nc.scalar.activation(gh[:, MS(m)], gpre[:], AF.Sigmoid)

            # ---- bilinear discriminators ----
            gate(10)
            gT = persist.tile([64, R], f16, tag="gT")
            gaT = persist.tile([64, R], f16, tag="gaT")
            with tc.tile_pool(name="pt3", bufs=4, space="PSUM") as pt:
                for m in range(4):
                    for (dst_, lo) in ((gT, 0), (gaT, 64)):
                        tp = pt.tile([64, 128], f16, tag="tb64")
                        nc.tensor.transpose(
                            tp[:], gh[:, 128 * m + lo:128 * m + lo + 64],
                            ident[:])
                        nc.vector.tensor_copy(dst_[:, MS(m)], tp[:])
            with tc.tile_pool(name="pq", bufs=4, space="PSUM") as pq:
                for m in range(4):
                    qp = pq.tile([128, DOUT], f32, tag="qp")
                    nc.tensor.matmul(qp[:], gT[:, MS(m)], WbTt[:],
                                     start=True, stop=True)
                    qap = pq.tile([128, DOUT], f32, tag="qp")
                    nc.tensor.matmul(qap[:], gaT[:, MS(m)], WbTt[:],
                                     start=True, stop=True)
                    scr = work.tile([128, DOUT], f32, tag="scr")
                    retp = work.tile([128, 2], f32, tag="retp")
                    retap = work.tile([128, 2], f32, tag="retap")
                    for (q_, e_, acc_) in (
                            (qp, embf32[:, m, 0:64], retp[:, 0:1]),
                            (qp, embf32[:, m, 64:128], retp[:, 1:2]),
                            (qap, embf32[:, m, 64:128], retap[:, 0:1]),
                            (qap, embf32[:, m, 0:64], retap[:, 1:2])):
                        nc.vector.tensor_tensor(scr[:], q_[:], e_, op=OP.mult)
                        nc.vector.tensor_reduce(acc_, scr[:], axis=X, op=OP.add)
                    nc.scalar.activation(retp[:], retp[:], AF.Sigmoid,
                                         bias=bbt[:])
                    nc.scalar.activation(retap[:], retap[:], AF.Sigmoid,
                                         bias=bbt[:])
                    nc.sync.dma_start(ret_out[MS(m), :], retp[:])
                    nc.sync.dma_start(reta_out[MS(m), :], retap[:])

            # ---- L2 blocks: gather, scores, aggregation ----
            gate(11)
            with tc.tile_pool(name="pagg2", bufs=1, space="PSUM") as pagg2:
                agg2 = pagg2.tile([65, R], f32, tag="agg2")
                for b in range(NBLK):
                    isl = slice((EB // 16) * b, (EB // 16) * (b + 1))
                    gb = gath.tile([128, BLK, 640], f16, tag="gslot",
                                   name=f"gb{b}")
                    nc.gpsimd.dma_gather(gb[:], t2bloc[:, 0:640],
                                         dstlwt[:, isl], EB, EB, 640,
                                         elem_step=640)
                    gz = eep.tile([128, BLK, 128], f16, tag="gz")
                    nc.gpsimd.dma_gather(gz[:], embfull[:, 128:256],
                                         srct[:, isl], EB, EB, 128,
                                         elem_step=256)
                    ga = gath.tile([128, BLK, DIN], f16, tag="gslot",
                                   name=f"ga{b}")
                    nc.gpsimd.dma_gather(ga[:], t2full[:, 0:DIN], srct[:, isl],
                                         EB, EB, DIN, elem_step=DIN)
                    # u2 in place into ga
                    nc.vector.tensor_tensor(ga[:], ga[:], gb[:, :, 0:512],
                                            op=OP.add)
                    pos = eep.tile([128, BLK], f32, tag="pos")
                    neg = eep.tile([128, BLK], f32, tag="neg")
                    nc.vector.tensor_reduce(pos[:], ga[:, :, 0:nposc],
                                            axis=X, op=OP.add,
                                            apply_absolute_value=True)
                    nc.vector.tensor_reduce(neg[:], ga[:, :, nposc:512],
                                            axis=X, op=OP.add,
                                            apply_absolute_value=True)
                    p2sum = eep.tile([128, BLK], f32, tag="p2sum")
                    nc.vector.tensor_tensor(
                        p2sum[:],
                        gz[:, :, 65:66].rearrange("p c one -> p (c one)"),
                        gb[:, :, 512:513].rearrange("p c one -> p (c one)"),
                        op=OP.add)
                    nc.vector.tensor_tensor(pos[:], pos[:], neg[:],
                                            op=OP.subtract)
                    e2 = eep.tile([128, BLK], f32, tag="e2")
                    nc.vector.scalar_tensor_tensor(e2[:], pos[:], 0.4,
                                                   p2sum[:], op0=OP.mult,
                                                   op1=OP.add)
                    nc.scalar.activation(e2[:], e2[:], AF.Exp)
                    nc.vector.tensor_tensor(
                        e2[:], e2[:],
                        emaskt[:, BLK * b:BLK * (b + 1), :].rearrange(
                            "p c one -> p (c one)"),
                        op=OP.mult)
                    eeh2 = eep.tile([128, BLK, 1], f16, tag="eeh2")
                    nc.vector.tensor_copy(
                        eeh2[:].rearrange("p c one -> p (c one)"), e2[:])
                    V2 = vp.tile([128, BLK, 65], f16, tag="v2slot",
                                 name=f"V2_{b}")
                    nc.vector.tensor_tensor(
                        V2[:, :, 0:64], gz[:, :, 1:65],
                        eeh2[:].to_broadcast([128, BLK, 64]), op=OP.mult)
                    nc.vector.tensor_copy(V2[:, :, 64:65], eeh2[:])
                    HT = build_HT(b, "L2")
                    for c in range(BLK):
                        cc = BLK * b + c
                        nc.tensor.matmul(agg2[:], V2[:, c, :], HT[:, c, :],
                                         start=(cc == 0),
                                         stop=(cc == NCHUNK - 1))
                a2 = work.tile([65, R], f32, tag="aggev", bufs=2)
                nc.vector.tensor_copy(a2[:], agg2[:])
            # normalize per node (transpose -> per-partition recip),
            # write node rows for the aggn allgather
            with tc.tile_pool(name="pt2a", bufs=4, space="PSUM") as pt:
                for m in range(4):
                    tp = pt.tile([128, 65], f32, tag="tp65")
                    nc.tensor.transpose(tp[:], a2[:, MS(m)], idf32[0:65, 0:65])
                    rsr = work.tile([128, 1], f32, tag="rsr")
                    nc.vector.tensor_scalar_max(rsr[:], tp[:, 64:65], 1e-30)
                    nc.vector.reciprocal(rsr[:], rsr[:])
                    agn = work.tile([128, 64], f16, tag="agn")
                    nc.vector.tensor_scalar_mul(agn[:], tp[:, 0:64], rsr[:])
                    nc.sync.dma_start(aggnloc[MS(m), :], agn[:])
            AG(aggnloc.ap(), aggnfull.ap())

            # ---- h = (adj @ aggn) @ Wl2: yT then 4 small matmuls ----
            gate(12)
            yT = persist.tile([64, R], f16, tag="yT")
            with tc.tile_pool(name="py", bufs=1, space="PSUM") as py:
                yps = py.tile([64, R], f32, tag="yps")
                for g in range(8):
                    l = work.tile([128, 4, DOUT], f16, tag="agld", bufs=2)
                    nc.sync.dma_start(
                        l[:], aggnfull[512 * g:512 * (g + 1), :].rearrange(
                            "(j p) r -> p j r", p=128))
                    for j in range(4):
                        kc = 4 * g + j
                        nc.tensor.matmul(yps[:], l[:, j, :], adjTh[:, kc, :],
                                         start=(kc == 0), stop=(kc == 31))
                nc.vector.tensor_scalar_mul(yT[:], yps[:], 1.0 / ADJ_SCALE)
            gate(13)
            with tc.tile_pool(name="ph", bufs=2, space="PSUM") as ph:
                for m in range(4):
                    hp = ph.tile([128, DIN], f32, tag="hps")
                    nc.tensor.matmul(hp[:], yT[:, MS(m)], Wl2t[:],
                                     start=True, stop=True)
                    he = work.tile([128, DIN], f32, tag="hev", bufs=2)
                    nc.vector.tensor_copy(he[:], hp[:])
                    nc.sync.dma_start(h_out[MS(m), :], he[:])

    nc.compile()
    return nc


def _prep_inputs(inputs):
    """Host-side sharding + weight transforms. Returns per-core input maps."""
    feat = np.asarray(inputs["feat"], np.float32)
    feat_a = np.asarray(inputs["feat_a"], np.float32)
    adj = np.asarray(inputs["adj"], np.float32)
    gn = np.asarray(inputs["graph_neigh"], np.float32)
    src = np.asarray(inputs["src"], np.int32)
    dst = np.asarray(inputs["dst"], np.int32)
    Wl1 = np.asarray(inputs["Wl1"], np.float32)
    Wr1 = np.asarray(inputs["Wr1"], np.float32)
    att1 = np.asarray(inputs["att1"], np.float32)
    Wl2 = np.asarray(inputs["Wl2"], np.float32)
    Wr2 = np.asarray(inputs["Wr2"], np.float32)
    att2 = np.asarray(inputs["att2"], np.float32)
    Wb = np.asarray(inputs["Wb"], np.float32)
    bb = np.asarray(inputs["bb"], np.float32)

    # sign-split permutation for att2 (positives first)
    order = np.argsort((att2 <= 0).astype(np.int32), kind="stable")
    npos = int((att2 > 0).sum())
    _CACHE["npos"] = npos
    W2sl = (Wl2 * att2[None, :])[:, order].astype(np.float16)
    W2sr = (Wr2 * att2[None, :])[:, order].astype(np.float16)
    cl6 = (0.6 * (Wl2 @ att2))[:, None].astype(np.float16)
    cr6 = (0.6 * (Wr2 @ att2))[:, None].astype(np.float16)

    shared = {
        "Wl1h": Wl1.astype(np.float16),
        "Wr1h": Wr1.astype(np.float16),
        "att1r": np.broadcast_to(att1.astype(np.float16)[None, :],
                                 (128, DOUT)).copy(),
        "W2sl": W2sl, "W2sr": W2sr, "cl6": cl6, "cr6": cr6,
        "Wl2h": Wl2.astype(np.float16),
        "WbTh": Wb.T.astype(np.float16),
        "bbcol": np.full((128, 1), bb[0], np.float32),
        "iota512": np.broadcast_to(np.arange(DIN, dtype=np.float16)[None, :],
                                   (128, DIN)).copy(),
    }

    in_maps = []
    for r in range(NC):
        rows = slice(R * r, R * (r + 1))
        sel = np.where((dst >= R * r) & (dst < R * (r + 1)))[0]
        ne = len(sel)
        assert ne <= EP, f"edge overflow: {ne} > {EP}"
        es = np.zeros(EP, np.int32)
        dl = np.zeros(EP, np.int32)
        es[:ne] = src[sel]
        dl[:ne] = dst[sel] - R * r
        mask = np.zeros(EP, np.float16)
        mask[:ne] = 1.0
        # edge -> (partition e%128, chunk e//128) layout for per-edge scalars
        dl16 = dl.reshape(NCHUNK, 128).T.astype(np.float16)[:, :, None]
        m16 = mask.reshape(NCHUNK, 128).T[:, :, None]
        m = dict(shared)
        m.update({
            "featT": np.ascontiguousarray(feat[rows].T),
            "feataT": np.ascontiguousarray(feat_a[rows].T),
            "adjT16": np.ascontiguousarray(
                (adj[rows].T * ADJ_SCALE).astype(np.float16)),
            "gnT16": np.ascontiguousarray(gn[rows].T.astype(np.float16)),
            "srcw": _wrap_idx(es),
            "dstlw": _wrap_idx(dl),
            "dstl": np.ascontiguousarray(dl16),
            "emask": np.ascontiguousarray(m16),
        })
        in_maps.append(m)
    return in_maps


def kernel(**inputs):
    from concourse.bass_utils import run_bass_kernel_spmd

    in_maps = _prep_inputs(inputs)
    if "nc" not in _CACHE:
        _CACHE["nc"] = _build_program()
    nc = _CACHE["nc"]
    res = run_bass_kernel_spmd(nc, in_maps, list(range(NC)))
    outs = res.results
    hiden_emb = np.concatenate([outs[c]["hid_out"] for c in range(NC)], axis=0)
    h = np.concatenate([outs[c]["h_out"] for c in range(NC)], axis=0)
    ret = np.concatenate([outs[c]["ret_out"] for c in range(NC)], axis=0)
    ret_a = np.concatenate([outs[c]["reta_out"] for c in range(NC)], axis=0)
    return (hiden_emb, h, ret, ret_a)
